# revision 7
# baseline (speedup 1.0000x reference)
"""Trainium2 Bass kernel for CondTupleGPT (2-group GPT: 6+2 layers, D=512, H=8, T=1024, B=2).

Sharding: 8 cores = 2 batch groups x 4 sequence chunks (256 tokens each).
 - All weight matmuls are local per core (weights replicated, bf16).
 - Per layer, one AllGather of (K^T, V) within each 4-core batch group.
 - Residual stream lives in SBUF in fp32 for the whole network.
Layout strategy:
 - x: [tokens(part), D(free)]; LN reduces along free axis.
 - h^T: [D(part), tokens(free)] via PE transposes; feeds all matmuls.
 - scores computed transposed [tk, tq]; exp'd, causally masked by a
   per-core multiplicative mask input, then AV matmul consumes them directly.
 - softmax denominators from an augmented ones-column in V; normalization
   via K=1 broadcast matmul + elementwise multiply.
 - LN scale/shift folded into the following matmul weights on the host.
"""
import sys

sys.path.insert(0, '/opt/trn_rl_repo')

import numpy as np
import ml_dtypes

import concourse.bass as bass
import concourse.tile as tile
from concourse import bacc, mybir
from concourse import bass_utils
from concourse.masks import make_identity

bf16 = mybir.dt.bfloat16
f32 = mybir.dt.float32
i32 = mybir.dt.int32
AF = mybir.ActivationFunctionType
ALU = mybir.AluOpType

B, T, D, H = 2, 1024, 512, 8
HD = D // H
N_LAYERS = (6, 2)
NL = sum(N_LAYERS)
V0, V1 = 8192, 4097
V1P = 4608  # padded to 9*512
TLOC = 256  # tokens per core
NCHUNK = 4  # seq chunks per batch
P = 128
NTK = T // P  # 8 global key tiles
EPS = 1e-5

_CACHE = {}


def _build_program(flags):
    nc = bacc.Bacc("TRN2", target_bir_lowering=False, debug=False, num_devices=8)

    # ---------------- DRAM tensor declarations ----------------
    t_idx = nc.dram_tensor("idxs", [P, 2, 4], i32, kind="ExternalInput")
    t_pos = nc.dram_tensor("pos", [TLOC, D], f32, kind="ExternalInput")
    t_emb0 = nc.dram_tensor("emb0", [V0, D], f32, kind="ExternalInput")
    t_emb1 = nc.dram_tensor("emb1", [V1, D], f32, kind="ExternalInput")
    t_embx = nc.dram_tensor("embx", [512, D], f32, kind="ExternalInput")
    t_cmask = nc.dram_tensor("cmask", [P, NTK, TLOC], bf16, kind="ExternalInput")
    t_wA, t_w1, t_w2, t_bqk, t_b1 = [], [], [], [], []
    for l in range(NL):
        t_wA.append(nc.dram_tensor(f"wA{l}", [4, D, D], bf16, kind="ExternalInput"))
        t_w1.append(nc.dram_tensor(f"w1_{l}", [D, 4 * D], bf16, kind="ExternalInput"))
        t_w2.append(nc.dram_tensor(f"w2_{l}", [4 * D, D], bf16, kind="ExternalInput"))
        t_bqk.append(nc.dram_tensor(f"bqk{l}", [P, 8], f32, kind="ExternalInput"))
        t_b1.append(nc.dram_tensor(f"b1_{l}", [P, 16], f32, kind="ExternalInput"))
    t_wh0 = nc.dram_tensor("wh0", [D, V0], bf16, kind="ExternalInput")
    t_wh1 = nc.dram_tensor("wh1", [D, V1P], bf16, kind="ExternalInput")
    t_lg0 = nc.dram_tensor("lg0", [TLOC, V0], f32, kind="ExternalOutput")
    t_lg1 = nc.dram_tensor("lg1", [TLOC, V1P], f32, kind="ExternalOutput")

    with tile.TileContext(nc) as tc:
        import contextlib
        ctx = contextlib.ExitStack()
        with ctx:
            one = ctx.enter_context(tc.tile_pool(name="one", bufs=1))
            resid = ctx.enter_context(tc.tile_pool(name="resid", bufs=1))
            sln = ctx.enter_context(tc.tile_pool(name="sln", bufs=2))
            hTp = ctx.enter_context(tc.tile_pool(name="hT", bufs=2))
            qkp = ctx.enter_context(tc.tile_pool(name="qk", bufs=2))
            kvf = ctx.enter_context(tc.tile_pool(name="kvf", bufs=1))
            expp = ctx.enter_context(tc.tile_pool(name="expp", bufs=2))
            ytp = ctx.enter_context(tc.tile_pool(name="ytp", bufs=2))
            gelp = ctx.enter_context(tc.tile_pool(name="gelp", bufs=1))
            wa_p = ctx.enter_context(tc.tile_pool(name="wa", bufs=1))
            w1_p = ctx.enter_context(tc.tile_pool(name="w1", bufs=1))
            w2_p = ctx.enter_context(tc.tile_pool(name="w2", bufs=1))
            wh_p = ctx.enter_context(tc.tile_pool(name="wh", bufs=3))
            bias_p = ctx.enter_context(tc.tile_pool(name="bias", bufs=2))
            smol = ctx.enter_context(tc.tile_pool(name="smol", bufs=4))
            embp = ctx.enter_context(tc.tile_pool(name="embp", bufs=1))
            outp = ctx.enter_context(tc.tile_pool(name="outp", bufs=4))
            dram = ctx.enter_context(tc.tile_pool(name="dram", bufs=2, space="DRAM"))

            ps_tp = ctx.enter_context(tc.tile_pool(name="ps_tp", bufs=2, space="PSUM"))
            ps_mm = ctx.enter_context(tc.tile_pool(name="ps_mm", bufs=2, space="PSUM"))
            ps_sc = ctx.enter_context(tc.tile_pool(name="ps_sc", bufs=2, space="PSUM"))
            ps_yt = ctx.enter_context(tc.tile_pool(name="ps_yt", bufs=2, space="PSUM"))

            ident = one.tile([P, P], bf16)
            make_identity(nc, ident[:])
            ones64 = one.tile([1, 64], bf16)
            nc.vector.memset(ones64[:], 1.0)
            eps_t = one.tile([P, 1], f32)
            nc.vector.memset(eps_t[:], EPS)

            cmask = one.tile([P, NTK, TLOC], bf16)
            nc.sync.dma_start(cmask[:], t_cmask.ap())

            # ---------------- embeddings ----------------
            idxs = one.tile([P, 2, 4], i32)
            nc.sync.dma_start(idxs[:], t_idx.ap())
            x = resid.tile([P, 2, D], f32)
            post = embp.tile([P, 2, D], f32, tag="pos")
            nc.sync.dma_start(post[:], t_pos.ap().rearrange("(j p) n -> p j n", p=P))
            for j in range(2):
                g0 = embp.tile([P, D], f32, tag="g0")
                g1 = embp.tile([P, D], f32, tag="g1")
                gx = embp.tile([P, D], f32, tag="gx")
                nc.gpsimd.indirect_dma_start(
                    out=g0[:], out_offset=None, in_=t_emb0.ap(),
                    in_offset=bass.IndirectOffsetOnAxis(ap=idxs[:, j, 0:1], axis=0))
                nc.gpsimd.indirect_dma_start(
                    out=g1[:], out_offset=None, in_=t_emb1.ap(),
                    in_offset=bass.IndirectOffsetOnAxis(ap=idxs[:, j, 1:2], axis=0))
                nc.gpsimd.indirect_dma_start(
                    out=gx[:], out_offset=None, in_=t_embx.ap(),
                    in_offset=bass.IndirectOffsetOnAxis(ap=idxs[:, j, 2:3], axis=0))
                nc.vector.tensor_add(x[:, j, :], g0[:], g1[:])
                nc.vector.tensor_add(x[:, j, :], x[:, j, :], gx[:])
                nc.vector.tensor_add(x[:, j, :], x[:, j, :], post[:, j, :])

            # ---------------- helpers ----------------
            def layernorm_T(xt, tag):
                """LN (no affine: folded into next weights) -> transposed bf16 hT [P,4,TLOC]."""
                s = sln.tile([P, 2, D], bf16, tag="s")
                for j in range(2):
                    stats = smol.tile([P, 6], f32, tag="stats")
                    nc.vector.bn_stats(stats[:], xt[:, j, :])
                    mv = smol.tile([P, 2], f32, tag="mv")
                    nc.vector.bn_aggr(mv[:], stats[:])
                    rstd = smol.tile([P, 1], f32, tag="rstd")
                    nc.scalar.activation(rstd[:], mv[:, 1:2], AF.Sqrt, bias=eps_t[:])
                    nc.vector.reciprocal(rstd[:], rstd[:])
                    nc.vector.tensor_scalar(
                        s[:, j, :], xt[:, j, :], mv[:, 0:1], rstd[:],
                        op0=ALU.subtract, op1=ALU.mult)
                hT = hTp.tile([P, 4, TLOC], bf16, tag="hT")
                for j in range(2):
                    for kb in range(4):
                        tp = ps_tp.tile([P, P], bf16, tag="tp")
                        nc.tensor.transpose(tp[:], s[:, j, kb * P:(kb + 1) * P], ident[:])
                        nc.vector.tensor_copy(hT[:, kb, j * P:(j + 1) * P], tp[:])
                return hT

            def block(l):
                # weight loads
                wA = wa_p.tile([P, 4, 4, D], bf16, tag="wA")
                for w in range(4):
                    nc.sync.dma_start(
                        wA[:, :, w, :],
                        t_wA[l].ap()[w].rearrange("(ko p) n -> p ko n", p=P))
                w1 = w1_p.tile([P, 4, 4 * D], bf16, tag="w1")
                nc.sync.dma_start(w1[:], t_w1[l].ap().rearrange("(ko p) n -> p ko n", p=P))
                w2 = w2_p.tile([P, 16, D], bf16, tag="w2")
                nc.sync.dma_start(w2[:], t_w2[l].ap().rearrange("(ko p) n -> p ko n", p=P))
                bqk = bias_p.tile([P, 8], f32, tag="bqk")
                nc.sync.dma_start(bqk[:], t_bqk[l].ap())
                b1 = bias_p.tile([P, 16], f32, tag="b1")
                nc.sync.dma_start(b1[:], t_b1[l].ap())

                hT = layernorm_T(x, "hT1")

                # q,k: [od(part), t] with bias add
                qT = qkp.tile([P, 4, TLOC], bf16, tag="qT")
                kT = qkp.tile([P, 4, TLOC], bf16, tag="kT")
                for w, (dst, bcol) in enumerate([(qT, 0), (kT, 4)]):
                    for ot in range(4):
                        acc = ps_mm.tile([P, TLOC], f32, tag="mm")
                        for ko in range(4):
                            nc.tensor.matmul(
                                acc[:], wA[:, ko, w, ot * P:(ot + 1) * P], hT[:, ko, :],
                                start=(ko == 0), stop=(ko == 3))
                        nc.vector.tensor_scalar_add(
                            dst[:, ot, :], acc[:], bqk[:, bcol + ot:bcol + ot + 1])
                # v: [t(part), D]
                v_sb = qkp.tile([P, 2, D], bf16, tag="v")
                for ts in range(2):
                    acc = ps_mm.tile([P, D], f32, tag="mm")
                    for ko in range(4):
                        nc.tensor.matmul(
                            acc[:], hT[:, ko, ts * P:(ts + 1) * P], wA[:, ko, 2, :],
                            start=(ko == 0), stop=(ko == 3))
                    nc.vector.tensor_copy(v_sb[:, ts, :], acc[:])

                # ---- AllGather K^T and V across the 4-core batch group
                blk = D * TLOC  # elements per tensor per rank
                agi = dram.tile([2 * blk], bf16, tag="agi")
                # k part: flat[(a*128+p)*TLOC + t] = kT[p, a, t]
                nc.sync.dma_start(
                    agi[0:blk].rearrange("(a p t) -> p a t", p=P, a=4), kT[:])
                # v part: flat[blk + (j*128+p)*D + d] = v_sb[p, j, d]
                nc.sync.dma_start(
                    agi[blk:2 * blk].rearrange("(j p d) -> p j d", j=2, p=P), v_sb[:])
                ago = dram.tile([4, 2 * blk], bf16, tag="ago")
                nc.gpsimd.collective_compute(
                    "AllGather", ALU.bypass,
                    replica_groups=[[0, 1, 2, 3], [4, 5, 6, 7]],
                    ins=[agi[:].opt()], outs=[ago[:].opt()],
                )
                # kfull [p, dblk, r, t]: d=dblk*128+p; K^T stored [dblk,p,t] per rank
                kfull = kvf.tile([P, 4, 4, TLOC], bf16, tag="kfull")
                for r in range(4):
                    nc.sync.dma_start(
                        kfull[:, :, r, :],
                        ago[r, 0:blk].rearrange("(a p t) -> p a t", a=4, p=P))
                # vaug [p, j(tk tile 8), h, 65]; last col ones
                vaug = kvf.tile([P, 2 * 4, H, 65], bf16, tag="vaug")
                for r in range(4):
                    for jj in range(2):
                        nc.sync.dma_start(
                            vaug[:, 2 * r + jj, :, 0:64],
                            ago[r, blk + jj * P * D:blk + (jj + 1) * P * D].rearrange(
                                "(p h e) -> p h e", p=P, h=H))
                nc.gpsimd.memset(vaug[:, :, :, 64:65], 1.0)

                # ---- attention per head
                yT = ytp.tile([P, 4, TLOC], bf16, tag="yT")
                for h in range(H):
                    pbase, dblk = 64 * (h % 2), h // 2
                    yac = ps_yt.tile([P, TLOC], f32, tag="yac")
                    for j in range(NTK):
                        sc = ps_sc.tile([P, TLOC], f32, tag="sc")
                        nc.tensor.matmul(
                            sc[:], kfull[pbase:pbase + 64, dblk, j // 2,
                                         (j % 2) * P:(j % 2) * P + P],
                            qT[pbase:pbase + 64, dblk, :],
                            start=True, stop=True)
                        et = expp.tile([P, NTK, TLOC], bf16, tag="et")
                        nc.scalar.activation(et[:, j, :], sc[:], AF.Exp, scale=0.125)
                        nc.vector.tensor_mul(et[:, j, :], et[:, j, :], cmask[:, j, :])
                        nc.tensor.matmul(
                            yac[0:65, :], vaug[:, j, h, :], et[:, j, :],
                            start=(j == 0), stop=(j == NTK - 1))
                    # normalize by sums (row 64) and write packed yT
                    sumr = smol.tile([1, TLOC], f32, tag="sumr")
                    nc.vector.reciprocal(sumr[:], yac[64:65, :])
                    sumrb = smol.tile([1, TLOC], bf16, tag="sumrb")
                    nc.vector.tensor_copy(sumrb[:], sumr[:])
                    bcp = ps_sc.tile([64, TLOC], f32, tag="sc")
                    nc.tensor.matmul(bcp[:], ones64[:], sumrb[:], start=True, stop=True)
                    bcs = smol.tile([64, TLOC], f32, tag="bcs")
                    nc.vector.tensor_copy(bcs[:], bcp[:])
                    nc.vector.tensor_mul(
                        yT[pbase:pbase + 64, dblk, :], yac[0:64, :], bcs[:])

                # ---- proj + residual
                for ts in range(2):
                    acc = ps_mm.tile([P, D], f32, tag="mm")
                    for ko in range(4):
                        nc.tensor.matmul(
                            acc[:], yT[:, ko, ts * P:(ts + 1) * P], wA[:, ko, 3, :],
                            start=(ko == 0), stop=(ko == 3))
                    nc.vector.tensor_add(x[:, ts, :], x[:, ts, :], acc[:])

                # ---- MLP
                h2T = layernorm_T(x, "hT2")
                gel = gelp.tile([P, 16, TLOC], bf16, tag="gel")
                for ot in range(16):
                    acc = ps_mm.tile([P, TLOC], f32, tag="mm")
                    for ko in range(4):
                        nc.tensor.matmul(
                            acc[:], w1[:, ko, ot * P:(ot + 1) * P], h2T[:, ko, :],
                            start=(ko == 0), stop=(ko == 3))
                    nc.scalar.activation(gel[:, ot, :], acc[:], AF.Gelu,
                                         bias=b1[:, ot:ot + 1])
                for ts in range(2):
                    acc = ps_mm.tile([P, D], f32, tag="mm")
                    for ko in range(16):
                        nc.tensor.matmul(
                            acc[:], gel[:, ko, ts * P:(ts + 1) * P], w2[:, ko, :],
                            start=(ko == 0), stop=(ko == 15))
                    nc.vector.tensor_add(x[:, ts, :], x[:, ts, :], acc[:])

            def head(s, t_wh, t_lg, nslices):
                hfT = layernorm_T(x, "hTf")
                for ns in range(nslices):
                    wh = wh_p.tile([P, 4, 512], bf16, tag="wh")
                    nc.sync.dma_start(
                        wh[:],
                        t_wh.ap().rearrange("(ko p) v -> p ko v", p=P)[:, :, ns * 512:(ns + 1) * 512])
                    for ts in range(2):
                        acc = ps_mm.tile([P, 512], f32, tag="mm")
                        for ko in range(4):
                            nc.tensor.matmul(
                                acc[:], hfT[:, ko, ts * P:(ts + 1) * P], wh[:, ko, :],
                                start=(ko == 0), stop=(ko == 3))
                        ot = outp.tile([P, 512], f32, tag="lg")
                        nc.vector.tensor_copy(ot[:], acc[:])
                        nc.sync.dma_start(
                            t_lg.ap().rearrange("(j p) v -> p j v", p=P)[:, ts, ns * 512:(ns + 1) * 512],
                            ot[:])

            # ---------------- the network ----------------
            for l in range(N_LAYERS[0]):
                block(l)
            head(0, t_wh0, t_lg0, V0 // 512)
            # x += emb0[target_idx0]
            for j in range(2):
                gt = embp.tile([P, D], f32, tag="gt")
                nc.gpsimd.indirect_dma_start(
                    out=gt[:], out_offset=None, in_=t_emb0.ap(),
                    in_offset=bass.IndirectOffsetOnAxis(ap=idxs[:, j, 3:4], axis=0))
                nc.vector.tensor_add(x[:, j, :], x[:, j, :], gt[:])
            for l in range(N_LAYERS[0], NL):
                block(l)
            head(1, t_wh1, t_lg1, V1P // 512)

    nc.compile()
    return nc


def _prep_host(idx, extra_idx, L_cond, target_idx, params):
    """Host-side preprocessing: fold LN affines into weights, cast, shard."""
    g = lambda a: np.asarray(a)
    tok0, tok1 = g(params['tok_embs'][0]).astype(np.float32), g(params['tok_embs'][1]).astype(np.float32)
    tokx = g(params['extra_tok_embs'][0]).astype(np.float32)
    Lc = int(L_cond)
    pos_full = np.concatenate([
        g(params['cond_pos_emb'])[:Lc], g(params['pos_emb'])[:T - Lc]], 0).astype(np.float32)

    bf = ml_dtypes.bfloat16
    layers = []   # per layer dict of arrays
    heads = []
    for gi in range(2):
        grp = params['groups'][gi]
        bl = {k: g(v).astype(np.float32) for k, v in grp['blocks'].items()}
        L = bl['Wq'].shape[0]
        for li in range(L):
            w = {}
            g1, b1v = bl['ln1_g'][li], bl['ln1_b'][li]
            wq = g1[:, None] * bl['Wq'][li]
            wk = g1[:, None] * bl['Wk'][li]
            wv = g1[:, None] * bl['Wv'][li]
            bq = bl['bq'][li] + b1v @ bl['Wq'][li]
            bk = bl['bk'][li] + b1v @ bl['Wk'][li]
            bv = bl['bv'][li] + b1v @ bl['Wv'][li]
            assert np.abs(bv).max() == 0.0, "nonzero v bias not supported in this kernel"
            assert np.abs(bl['bp'][li]).max() == 0.0
            assert np.abs(bl['b2'][li]).max() == 0.0
            g2, b2v = bl['ln2_g'][li], bl['ln2_b'][li]
            w1 = g2[:, None] * bl['W1'][li]
            b1f = bl['b1'][li] + b2v @ bl['W1'][li]
            w['wA'] = np.stack([wq, wk, wv, bl['Wp'][li]], 0).astype(bf)
            w['w1'] = w1.astype(bf)
            w['w2'] = bl['W2'][li].astype(bf)
            # bqk [128, 8]: cols 0-3 bq od-tiles, 4-7 bk
            w['bqk'] = np.concatenate(
                [bq.reshape(4, P).T, bk.reshape(4, P).T], 1).astype(np.float32).copy()
            w['b1'] = b1f.reshape(16, P).T.astype(np.float32).copy()
            layers.append(w)
        hd = grp['head']
        lg_, lb_ = g(hd['ln_g']).astype(np.float32), g(hd['ln_b']).astype(np.float32)
        W = g(hd['W']).astype(np.float32)
        bh = lb_ @ W
        assert np.abs(bh).max() == 0.0, "nonzero head bias not supported"
        heads.append((lg_[:, None] * W))

    wh0 = heads[0].astype(bf)
    wh1 = np.zeros((D, V1P), np.float32)
    wh1[:, :V1] = heads[1]
    wh1 = wh1.astype(bf)

    idx = g(idx).astype(np.int64)
    eidx = g(extra_idx).astype(np.int64)
    tidx = g(target_idx).astype(np.int64)

    in_maps = []
    for c in range(8):
        b, qc = c // NCHUNK, c % NCHUNK
        t0, t1 = qc * TLOC, (qc + 1) * TLOC
        idxs = np.zeros((P, 2, 4), np.int32)
        for j in range(2):
            sl = slice(t0 + j * P, t0 + (j + 1) * P)
            idxs[:, j, 0] = idx[b, sl, 0]
            idxs[:, j, 1] = idx[b, sl, 1]
            idxs[:, j, 2] = eidx[b, sl, 0]
            idxs[:, j, 3] = tidx[b, sl, 0]
        # causal mask [P(tk), NTK(j), TLOC(tq)]: allow iff j*128+tk <= qc*256+tq
        tk = np.arange(P)[:, None, None]
        jj = np.arange(NTK)[None, :, None]
        tq = np.arange(TLOC)[None, None, :]
        cm = ((jj * P + tk) <= (qc * TLOC + tq)).astype(bf)
        m = {
            'idxs': idxs, 'pos': pos_full[t0:t1].copy(),
            'emb0': tok0, 'emb1': tok1, 'embx': tokx, 'cmask': cm,
            'wh0': wh0, 'wh1': wh1,
        }
        for l, w in enumerate(layers):
            m[f'wA{l}'] = w['wA']
            m[f'w1_{l}'] = w['w1']
            m[f'w2_{l}'] = w['w2']
            m[f'bqk{l}'] = w['bqk']
            m[f'b1_{l}'] = w['b1']
        in_maps.append(m)
    return in_maps


def kernel(idx, extra_idx, L_cond, target_idx, params):
    in_maps = _prep_host(idx, extra_idx, L_cond, target_idx, params)
    if 'prog' not in _CACHE:
        _CACHE['prog'] = _build_program({})
    nc = _CACHE['prog']
    res = bass_utils.run_bass_kernel_spmd(nc, in_maps, core_ids=list(range(8)))
    _CACHE['last_results'] = res
    lg0 = np.zeros((B, T, V0), np.float32)
    lg1 = np.zeros((B, T, V1), np.float32)
    for c in range(8):
        b, qc = c // NCHUNK, c % NCHUNK
        t0, t1 = qc * TLOC, (qc + 1) * TLOC
        lg0[b, t0:t1] = res.results[c]['lg0']
        lg1[b, t0:t1] = res.results[c]['lg1'][:, :V1]
    return (lg0, lg1)


# revision 17
# speedup vs baseline: 1.0589x; 1.0589x over previous
"""Trainium2 Bass kernel for CondTupleGPT (2-group GPT: 6+2 layers, D=512, H=8, T=1024, B=2).

Sharding: 8 cores = 2 batch groups x 4 sequence chunks (256 tokens each).
 - All weight matmuls are local per core (weights replicated, bf16).
 - Per layer, one AllGather of (K^T, V) within each 4-core batch group.
 - Residual stream lives in SBUF in fp32 for the whole network.
Layout strategy:
 - x: [tokens(part), D(free)]; LN reduces along free axis.
 - h^T: [D(part), tokens(free)] via PE transposes; feeds all matmuls.
 - scores computed transposed [tk, tq]; exp'd, causally masked by a
   per-core multiplicative mask input, then AV matmul consumes them directly.
 - softmax denominators from an augmented ones-column in V; normalization
   via K=1 broadcast matmul + elementwise multiply.
 - LN scale/shift folded into the following matmul weights on the host.
"""
import sys

sys.path.insert(0, '/opt/trn_rl_repo')

import numpy as np
import ml_dtypes

import concourse.bass as bass
import concourse.tile as tile
from concourse import bacc, mybir
from concourse import bass_utils
from concourse.masks import make_identity

bf16 = mybir.dt.bfloat16
f32 = mybir.dt.float32
i32 = mybir.dt.int32
AF = mybir.ActivationFunctionType
ALU = mybir.AluOpType

B, T, D, H = 2, 1024, 512, 8
HD = D // H
N_LAYERS = (6, 2)
NL = sum(N_LAYERS)
V0, V1 = 8192, 4097
V1P = 4608  # padded to 9*512
TLOC = 256  # tokens per core
NCHUNK = 4  # seq chunks per batch
P = 128
NTK = T // P  # 8 global key tiles
EPS = 1e-5

_CACHE = {}


def _build_program(flags):
    nc = bacc.Bacc("TRN2", target_bir_lowering=False, debug=False, num_devices=flags.get("ndev", 8))

    # ---------------- DRAM tensor declarations ----------------
    t_idx = nc.dram_tensor("idxs", [P, 2, 4], i32, kind="ExternalInput")
    t_pos = nc.dram_tensor("pos", [TLOC, D], f32, kind="ExternalInput")
    t_emb0 = nc.dram_tensor("emb0", [V0, D], f32, kind="ExternalInput")
    t_emb1 = nc.dram_tensor("emb1", [V1, D], f32, kind="ExternalInput")
    t_embx = nc.dram_tensor("embx", [512, D], f32, kind="ExternalInput")
    t_cmask = nc.dram_tensor("cmask", [P, NTK, TLOC], bf16, kind="ExternalInput")
    t_wA, t_w1, t_w2, t_bqk, t_b1 = [], [], [], [], []
    for l in range(NL):
        t_wA.append(nc.dram_tensor(f"wA{l}", [D, 4, D], bf16, kind="ExternalInput"))
        t_w1.append(nc.dram_tensor(f"w1_{l}", [D, 4 * D], bf16, kind="ExternalInput"))
        t_w2.append(nc.dram_tensor(f"w2_{l}", [4 * D, D], bf16, kind="ExternalInput"))
        t_bqk.append(nc.dram_tensor(f"bqk{l}", [P, 24], f32, kind="ExternalInput"))
    t_wh0 = nc.dram_tensor("wh0", [D, V0], bf16, kind="ExternalInput")
    t_wh1 = nc.dram_tensor("wh1", [D, V1P], bf16, kind="ExternalInput")
    t_lg0 = nc.dram_tensor("lg0", [TLOC, V0], f32, kind="ExternalOutput")
    t_lg1 = nc.dram_tensor("lg1", [TLOC, V1P], f32, kind="ExternalOutput")

    with tile.TileContext(nc) as tc:
        import contextlib
        ctx = contextlib.ExitStack()
        with ctx:
            one = ctx.enter_context(tc.tile_pool(name="one", bufs=1))
            resid = ctx.enter_context(tc.tile_pool(name="resid", bufs=1))
            sln = ctx.enter_context(tc.tile_pool(name="sln", bufs=2))
            hTp = ctx.enter_context(tc.tile_pool(name="hT", bufs=2))
            qkp = ctx.enter_context(tc.tile_pool(name="qk", bufs=2))
            kvf = ctx.enter_context(tc.tile_pool(name="kvf", bufs=1))
            expp = ctx.enter_context(tc.tile_pool(name="expp", bufs=5))
            ytp = ctx.enter_context(tc.tile_pool(name="ytp", bufs=2))
            gelp = ctx.enter_context(tc.tile_pool(name="gelp", bufs=1))
            wa_p = ctx.enter_context(tc.tile_pool(name="wa", bufs=1))
            w1_p = ctx.enter_context(tc.tile_pool(name="w1", bufs=1))
            w2_p = ctx.enter_context(tc.tile_pool(name="w2", bufs=1))
            wh_p = ctx.enter_context(tc.tile_pool(name="wh", bufs=3))
            bias_p = ctx.enter_context(tc.tile_pool(name="bias", bufs=2))
            smol = ctx.enter_context(tc.tile_pool(name="smol", bufs=4))
            embp = ctx.enter_context(tc.tile_pool(name="embp", bufs=1))
            outp = ctx.enter_context(tc.tile_pool(name="outp", bufs=2))
            dram = ctx.enter_context(tc.tile_pool(name="dram", bufs=2, space="DRAM"))

            ps_tp = ctx.enter_context(tc.tile_pool(name="ps_tp", bufs=1, space="PSUM"))
            ps_mm = ctx.enter_context(tc.tile_pool(name="ps_mm", bufs=2, space="PSUM"))
            ps_sc = ctx.enter_context(tc.tile_pool(name="ps_sc", bufs=4, space="PSUM"))
            ps_yt = ctx.enter_context(tc.tile_pool(name="ps_yt", bufs=1, space="PSUM"))

            ident = one.tile([P, P], bf16)
            make_identity(nc, ident[:])
            ones64 = one.tile([1, 64], bf16)
            nc.vector.memset(ones64[:], 1.0)
            eps_t = one.tile([P, 1], f32)
            nc.vector.memset(eps_t[:], EPS)

            cmask = one.tile([P, NTK, TLOC], bf16)
            nc.sync.dma_start(cmask[:], t_cmask.ap())

            # ---------------- embeddings ----------------
            idxs = one.tile([P, 2, 4], i32)
            nc.sync.dma_start(idxs[:], t_idx.ap())
            x = resid.tile([P, 2, D], f32)
            post = embp.tile([P, 2, D], f32, tag="pos")
            nc.sync.dma_start(post[:], t_pos.ap().rearrange("(j p) n -> p j n", p=P))
            for j in range(2):
                g0 = embp.tile([P, D], f32, tag="g0")
                g1 = embp.tile([P, D], f32, tag="g1")
                gx = embp.tile([P, D], f32, tag="gx")
                nc.gpsimd.indirect_dma_start(
                    out=g0[:], out_offset=None, in_=t_emb0.ap(),
                    in_offset=bass.IndirectOffsetOnAxis(ap=idxs[:, j, 0:1], axis=0))
                nc.gpsimd.indirect_dma_start(
                    out=g1[:], out_offset=None, in_=t_emb1.ap(),
                    in_offset=bass.IndirectOffsetOnAxis(ap=idxs[:, j, 1:2], axis=0))
                nc.gpsimd.indirect_dma_start(
                    out=gx[:], out_offset=None, in_=t_embx.ap(),
                    in_offset=bass.IndirectOffsetOnAxis(ap=idxs[:, j, 2:3], axis=0))
                nc.vector.tensor_add(x[:, j, :], g0[:], g1[:])
                nc.vector.tensor_add(x[:, j, :], x[:, j, :], gx[:])
                nc.vector.tensor_add(x[:, j, :], x[:, j, :], post[:, j, :])

            # ---------------- helpers ----------------
            def layernorm_T(xt, tag):
                """LN (no affine: folded into next weights) -> transposed bf16 hT [P,4,TLOC]."""
                s = sln.tile([P, 2, D], bf16, tag="s")
                for j in range(2):
                    stats = smol.tile([P, 6], f32, tag="stats")
                    nc.vector.bn_stats(stats[:], xt[:, j, :])
                    mv = smol.tile([P, 2], f32, tag="mv")
                    nc.vector.bn_aggr(mv[:], stats[:])
                    rstd = smol.tile([P, 1], f32, tag="rstd")
                    nc.scalar.activation(rstd[:], mv[:, 1:2], AF.Sqrt, bias=eps_t[:])
                    nc.vector.reciprocal(rstd[:], rstd[:])
                    nc.vector.tensor_scalar(
                        s[:, j, :], xt[:, j, :], mv[:, 0:1], rstd[:],
                        op0=ALU.subtract, op1=ALU.mult)
                hT = hTp.tile([P, 4, TLOC], bf16, tag="hT")
                if flags.get('dma_tr'):
                    for j in range(2):
                        for kb in range(4):
                            nc.sync.dma_start_transpose(
                                hT[:, kb, j * P:(j + 1) * P], s[:, j, kb * P:(kb + 1) * P])
                else:
                    for j in range(2):
                        for kb in range(4):
                            tp = ps_tp.tile([P, P], bf16, tag="tp")
                            nc.tensor.transpose(tp[:], s[:, j, kb * P:(kb + 1) * P], ident[:])
                            nc.vector.tensor_copy(hT[:, kb, j * P:(j + 1) * P], tp[:])
                return hT

            def block(l):
                # weight loads
                wA = wa_p.tile([P, 4, 4, D], bf16, tag="wA")
                nc.sync.dma_start(
                    wA[:], t_wA[l].ap().rearrange("(ko p) w n -> p ko w n", p=P))
                w1 = w1_p.tile([P, 4, 4 * D], bf16, tag="w1")
                nc.sync.dma_start(w1[:], t_w1[l].ap().rearrange("(ko p) n -> p ko n", p=P))
                w2 = w2_p.tile([P, 16, D], bf16, tag="w2")
                nc.sync.dma_start(w2[:], t_w2[l].ap().rearrange("(ko p) n -> p ko n", p=P))
                bqk = bias_p.tile([P, 24], f32, tag="bqk")
                nc.sync.dma_start(bqk[:], t_bqk[l].ap())
                b1 = bqk[:, 8:24]

                hT = layernorm_T(x, "hT1")

                # q,k: [od(part), t] with bias add
                qT = qkp.tile([P, 4, TLOC], bf16, tag="qT")
                kT = qkp.tile([P, 4, TLOC], bf16, tag="kT")
                for w, (dst, bcol) in enumerate([(qT, 0), (kT, 4)]):
                    for ot in range(4):
                        acc = ps_mm.tile([P, TLOC], f32, tag="mm")
                        for ko in range(4):
                            nc.tensor.matmul(
                                acc[:], wA[:, ko, w, ot * P:(ot + 1) * P], hT[:, ko, :],
                                start=(ko == 0), stop=(ko == 3))
                        nc.vector.tensor_scalar_add(
                            dst[:, ot, :], acc[:], bqk[:, bcol + ot:bcol + ot + 1])
                # v: [t(part), D] augmented with a ones column per head (8 heads x 65)
                v_sb = qkp.tile([P, 2, H, 65], bf16, tag="v")
                nc.gpsimd.memset(v_sb[:, :, :, 64:65], 1.0)
                for ts in range(2):
                    acc = ps_mm.tile([P, D], f32, tag="mm")
                    for ko in range(4):
                        nc.tensor.matmul(
                            acc[:], hT[:, ko, ts * P:(ts + 1) * P], wA[:, ko, 2, :],
                            start=(ko == 0), stop=(ko == 3))
                    nc.vector.tensor_copy(
                        v_sb[:, ts, :, 0:64],
                        acc[:].rearrange("p (h e) -> p h e", h=H))

                # ---- AllGather K^T and V_aug across the 4-core batch group
                blk = D * TLOC          # k elements per rank
                blkv = TLOC * H * 65    # augmented v elements per rank
                agi = dram.tile([blk + blkv], bf16, tag="agi")
                # k part: flat[(a*128+p)*TLOC + t] = kT[p, a, t]
                nc.sync.dma_start(
                    agi[0:blk].rearrange("(a p t) -> p a t", p=P, a=4), kT[:])
                # v part: rows of 520: flat[blk + (j*128+p)*520 + c] = v_sb[p, j, c]
                nc.sync.dma_start(
                    agi[blk:blk + blkv].rearrange("(j p c) -> p j c", j=2, p=P), v_sb[:])
                ago = dram.tile([4, blk + blkv], bf16, tag="ago")
                if flags.get('fake_ag'):
                    for r in range(4):
                        nc.sync.dma_start(ago[r, :], agi[:])
                else:
                    nc.gpsimd.collective_compute(
                        "AllGather", ALU.bypass,
                        replica_groups=[[0, 1, 2, 3], [4, 5, 6, 7]],
                        ins=[agi[:].opt()], outs=[ago[:].opt()],
                    )
                # kfull [p, dblk, r, t]: d=dblk*128+p; K^T stored [dblk,p,t] per rank
                kfull = kvf.tile([P, 4, 4, TLOC], bf16, tag="kfull")
                for r in range(4):
                    nc.sync.dma_start(
                        kfull[:, :, r, :],
                        ago[r, 0:blk].rearrange("(a p t) -> p a t", a=4, p=P))
                # vaug [p, j(tk tile 8), h, 65]; last col ones
                vaug = kvf.tile([P, 2 * 4, H, 65], bf16, tag="vaug")
                for r in range(4):
                    nc.sync.dma_start(
                        vaug[:, 2 * r:2 * r + 2, :, :],
                        ago[r, blk:blk + blkv].rearrange("(j p c) -> p j c", j=2, p=P))
                nc.gpsimd.memset(vaug[:, :, :, 64:65], 1.0)

                # ---- attention per head
                yT = ytp.tile([P, 4, TLOC], bf16, tag="yT")
                if flags.get('no_attn'):
                    nc.gpsimd.memset(yT[:], 0.0)
                for h in ([] if flags.get('no_attn') else range(H)):
                    pbase, dblk = 64 * (h % 2), h // 2
                    yac = ps_yt.tile([P, TLOC], f32, tag="yac")
                    ets = []
                    # phase 1: all score pairs -> exp -> mask (PE streams scores
                    # back-to-back; ACT/DVE pipeline behind)
                    for jp in range(NTK // 2):
                        sc = ps_sc.tile([P, 2 * TLOC], f32, tag="sc")
                        for jh in range(2):
                            j = 2 * jp + jh
                            nc.tensor.matmul(
                                sc[:, jh * TLOC:(jh + 1) * TLOC],
                                kfull[pbase:pbase + 64, dblk, j // 2,
                                      (j % 2) * P:(j % 2) * P + P],
                                qT[pbase:pbase + 64, dblk, :],
                                start=True, stop=True)
                        et = expp.tile([P, 2, TLOC], bf16, tag="et", name=f"et{h}_{jp}")
                        nc.scalar.activation(
                            et[:].rearrange("p a t -> p (a t)"), sc[:],
                            AF.Exp, scale=0.125)
                        nc.vector.tensor_mul(et[:], et[:], cmask[:, 2 * jp:2 * jp + 2, :])
                        ets.append(et)
                    # phase 2: AV accumulation
                    for jp in range(NTK // 2):
                        for jh in range(2):
                            j = 2 * jp + jh
                            nc.tensor.matmul(
                                yac[0:65, :], vaug[:, j, h, :], ets[jp][:, jh, :],
                                start=(j == 0), stop=(j == NTK - 1))
                    # normalize by sums (row 64) and write packed yT
                    sumr = smol.tile([1, TLOC], f32, tag="sumr")
                    nc.vector.reciprocal(sumr[:], yac[64:65, :])
                    sumrb = smol.tile([1, TLOC], bf16, tag="sumrb")
                    nc.vector.tensor_copy(sumrb[:], sumr[:])
                    bcp = ps_sc.tile([64, TLOC], f32, tag="sc")
                    nc.tensor.matmul(bcp[:], ones64[:], sumrb[:], start=True, stop=True)
                    bcs = smol.tile([64, TLOC], f32, tag="bcs")
                    nc.vector.tensor_copy(bcs[:], bcp[:])
                    nc.vector.tensor_mul(
                        yT[pbase:pbase + 64, dblk, :], yac[0:64, :], bcs[:])

                # ---- proj + residual
                for ts in range(2):
                    acc = ps_mm.tile([P, D], f32, tag="mm")
                    for ko in range(4):
                        nc.tensor.matmul(
                            acc[:], yT[:, ko, ts * P:(ts + 1) * P], wA[:, ko, 3, :],
                            start=(ko == 0), stop=(ko == 3))
                    nc.vector.tensor_add(x[:, ts, :], x[:, ts, :], acc[:])

                # ---- MLP
                if flags.get('no_mlp'):
                    return
                h2T = layernorm_T(x, "hT2")
                gel = gelp.tile([P, 16, TLOC], bf16, tag="gel")
                for ot in range(16):
                    acc = ps_mm.tile([P, TLOC], f32, tag="mm")
                    for ko in range(4):
                        nc.tensor.matmul(
                            acc[:], w1[:, ko, ot * P:(ot + 1) * P], h2T[:, ko, :],
                            start=(ko == 0), stop=(ko == 3))
                    nc.scalar.activation(gel[:, ot, :], acc[:], AF.Gelu,
                                         bias=b1[:, ot:ot + 1])
                for ts in range(2):
                    acc = ps_mm.tile([P, D], f32, tag="mm")
                    for ko in range(16):
                        nc.tensor.matmul(
                            acc[:], gel[:, ko, ts * P:(ts + 1) * P], w2[:, ko, :],
                            start=(ko == 0), stop=(ko == 15))
                    nc.vector.tensor_add(x[:, ts, :], x[:, ts, :], acc[:])

            def head(s, t_wh, t_lg, nslices):
                hfT = layernorm_T(x, "hTf")
                for g in range(0, nslices, 4):
                    gn = min(4, nslices - g)
                    ots = [outp.tile([P, 4, 512], f32, tag=f"lg{ts}", name=f"lgbuf{ts}") for ts in range(2)]
                    for ns in range(g, g + gn):
                        wh = wh_p.tile([P, 4, 512], bf16, tag="wh")
                        nc.sync.dma_start(
                            wh[:],
                            t_wh.ap().rearrange("(ko p) v -> p ko v", p=P)[:, :, ns * 512:(ns + 1) * 512])
                        for ts in range(2):
                            acc = ps_mm.tile([P, 512], f32, tag="mm")
                            for ko in range(4):
                                nc.tensor.matmul(
                                    acc[:], hfT[:, ko, ts * P:(ts + 1) * P], wh[:, ko, :],
                                    start=(ko == 0), stop=(ko == 3))
                            nc.vector.tensor_copy(ots[ts][:, ns - g, :], acc[:])
                    for ts in range(2):
                        nc.sync.dma_start(
                            t_lg.ap().rearrange("(j p) v -> p j v", p=P)[:, ts, g * 512:(g + gn) * 512],
                            ots[ts][:, 0:gn, :])

            # ---------------- the network ----------------
            for _rep in range(flags.get('reps', 1)):
                for l in range(N_LAYERS[0]):
                    block(l)
                head(0, t_wh0, t_lg0, V0 // 512)
                # x += emb0[target_idx0]
                for j in range(2):
                    gt = embp.tile([P, D], f32, tag="gt")
                    nc.gpsimd.indirect_dma_start(
                        out=gt[:], out_offset=None, in_=t_emb0.ap(),
                        in_offset=bass.IndirectOffsetOnAxis(ap=idxs[:, j, 3:4], axis=0))
                    nc.vector.tensor_add(x[:, j, :], x[:, j, :], gt[:])
                for l in range(N_LAYERS[0], NL):
                    block(l)
                head(1, t_wh1, t_lg1, V1P // 512)

    nc.compile()
    return nc


def _prep_host(idx, extra_idx, L_cond, target_idx, params):
    """Host-side preprocessing: fold LN affines into weights, cast, shard."""
    g = lambda a: np.asarray(a)
    tok0, tok1 = g(params['tok_embs'][0]).astype(np.float32), g(params['tok_embs'][1]).astype(np.float32)
    tokx = g(params['extra_tok_embs'][0]).astype(np.float32)
    Lc = int(L_cond)
    pos_full = np.concatenate([
        g(params['cond_pos_emb'])[:Lc], g(params['pos_emb'])[:T - Lc]], 0).astype(np.float32)

    bf = ml_dtypes.bfloat16
    layers = []   # per layer dict of arrays
    heads = []
    for gi in range(2):
        grp = params['groups'][gi]
        bl = {k: g(v).astype(np.float32) for k, v in grp['blocks'].items()}
        L = bl['Wq'].shape[0]
        for li in range(L):
            w = {}
            g1, b1v = bl['ln1_g'][li], bl['ln1_b'][li]
            wq = g1[:, None] * bl['Wq'][li]
            wk = g1[:, None] * bl['Wk'][li]
            wv = g1[:, None] * bl['Wv'][li]
            bq = bl['bq'][li] + b1v @ bl['Wq'][li]
            bk = bl['bk'][li] + b1v @ bl['Wk'][li]
            bv = bl['bv'][li] + b1v @ bl['Wv'][li]
            assert np.abs(bv).max() == 0.0, "nonzero v bias not supported in this kernel"
            assert np.abs(bl['bp'][li]).max() == 0.0
            assert np.abs(bl['b2'][li]).max() == 0.0
            g2, b2v = bl['ln2_g'][li], bl['ln2_b'][li]
            w1 = g2[:, None] * bl['W1'][li]
            b1f = bl['b1'][li] + b2v @ bl['W1'][li]
            w['wA'] = np.stack([wq, wk, wv, bl['Wp'][li]], 1).astype(bf)
            w['w1'] = w1.astype(bf)
            w['w2'] = bl['W2'][li].astype(bf)
            # bqk [128, 24]: cols 0-3 bq od-tiles, 4-7 bk, 8-23 b1
            w['bqk'] = np.concatenate(
                [bq.reshape(4, P).T, bk.reshape(4, P).T,
                 b1f.reshape(16, P).T], 1).astype(np.float32).copy()
            layers.append(w)
        hd = grp['head']
        lg_, lb_ = g(hd['ln_g']).astype(np.float32), g(hd['ln_b']).astype(np.float32)
        W = g(hd['W']).astype(np.float32)
        bh = lb_ @ W
        assert np.abs(bh).max() == 0.0, "nonzero head bias not supported"
        heads.append((lg_[:, None] * W))

    wh0 = heads[0].astype(bf)
    wh1 = np.zeros((D, V1P), np.float32)
    wh1[:, :V1] = heads[1]
    wh1 = wh1.astype(bf)

    idx = g(idx).astype(np.int64)
    eidx = g(extra_idx).astype(np.int64)
    tidx = g(target_idx).astype(np.int64)

    in_maps = []
    for c in range(8):
        b, qc = c // NCHUNK, c % NCHUNK
        t0, t1 = qc * TLOC, (qc + 1) * TLOC
        idxs = np.zeros((P, 2, 4), np.int32)
        for j in range(2):
            sl = slice(t0 + j * P, t0 + (j + 1) * P)
            idxs[:, j, 0] = idx[b, sl, 0]
            idxs[:, j, 1] = idx[b, sl, 1]
            idxs[:, j, 2] = eidx[b, sl, 0]
            idxs[:, j, 3] = tidx[b, sl, 0]
        # causal mask [P(tk), NTK(j), TLOC(tq)]: allow iff j*128+tk <= qc*256+tq
        tk = np.arange(P)[:, None, None]
        jj = np.arange(NTK)[None, :, None]
        tq = np.arange(TLOC)[None, None, :]
        cm = ((jj * P + tk) <= (qc * TLOC + tq)).astype(bf)
        m = {
            'idxs': idxs, 'pos': pos_full[t0:t1].copy(),
            'emb0': tok0, 'emb1': tok1, 'embx': tokx, 'cmask': cm,
            'wh0': wh0, 'wh1': wh1,
        }
        for l, w in enumerate(layers):
            m[f'wA{l}'] = w['wA']
            m[f'w1_{l}'] = w['w1']
            m[f'w2_{l}'] = w['w2']
            m[f'bqk{l}'] = w['bqk']
        in_maps.append(m)
    return in_maps


def kernel(idx, extra_idx, L_cond, target_idx, params):
    in_maps = _prep_host(idx, extra_idx, L_cond, target_idx, params)
    if 'prog' not in _CACHE:
        _CACHE['prog'] = _build_program({})
    nc = _CACHE['prog']
    res = bass_utils.run_bass_kernel_spmd(nc, in_maps, core_ids=list(range(8)))
    _CACHE['last_results'] = res
    lg0 = np.zeros((B, T, V0), np.float32)
    lg1 = np.zeros((B, T, V1), np.float32)
    for c in range(8):
        b, qc = c // NCHUNK, c % NCHUNK
        t0, t1 = qc * TLOC, (qc + 1) * TLOC
        lg0[b, t0:t1] = res.results[c]['lg0']
        lg1[b, t0:t1] = res.results[c]['lg1'][:, :V1]
    return (lg0, lg1)
